# revision 13
# baseline (speedup 1.0000x reference)
"""Trainium2 Bass kernel for DynamicGRU.

Problem: x [1024, 200, 512] fp32, GRU with H=512.
  gi = x @ W_ih.T + b_ih
  per step: gh = h @ W_hh.T + b_hh
            r = sigmoid(gi_r + gh_r); i = sigmoid(gi_i + gh_i)
            n = tanh(gi_n + r * gh_n)
            h = n + i * (h - n)
Returns (outs [B,S,H], h_last [B,H]).

Data-parallel over 8 cores (128 batch rows each).  Key structure:
  - Matmul operands are bf16 (1 col/cycle PE stream; fp32 and fp32r moving
    operands stream at half rate).  PSUM accumulation is fp32 and the carried
    state h plus gate tensors stay fp32(+fp32r), so only the PE sees bf16.
  - The host pre-transposes x to [S, part, K-chunk, B] bf16 and pre-packs
    W^T bf16 — the kernel never transposes x or W on-chip.
  - gi and gh accumulate into the SAME PSUM bank per gate (r/i), so the
    elementwise adds vanish; the n gate keeps x-side and h-side banks apart
    because r multiplies only the h side.  Biases enter each bank via a K=1
    ones-row outer-product matmul that opens the accumulation group.
  - h is carried as fp32r so its PE transpose (needed to feed the next
    recurrent matmul) runs single-pass; hT is rounded to bf16 in the
    PSUM->SBUF copy.
  - Gate nonlinearities on ScalarE, the rest of the chain on DVE in
    dependency order, m1 = i*h on GPSIMD, all chunked in halves so the PE
    transposes of new h start before the full lerp finishes.
"""

import numpy as np
import ml_dtypes

import concourse.bass as bass
import concourse.mybir as mybir
import concourse.tile as tile
from concourse import bacc
from concourse.bass_utils import run_bass_kernel_spmd
from concourse.masks import make_identity

B_FULL = 1024
NCORES = 8
B = B_FULL // NCORES  # 128 per core
S = 200
I = 512
H = 512
G3 = 3 * H

F32 = mybir.dt.float32
F32R = mybir.dt.float32r
BF16 = mybir.dt.bfloat16
AF = mybir.ActivationFunctionType
BF = ml_dtypes.bfloat16


def build(nc, seq_len=S):
    # Host-prepped inputs: xt[t, p, k, b] = x[b, t, 128k+p] in bf16;
    # wT = W.T (contraction-major) in bf16; biases pre-combined bf16.
    xt_d = nc.dram_tensor("xt", [seq_len, 128, 4, B], BF16, kind="ExternalInput")
    wihT_d = nc.dram_tensor("wihT", [I, G3], BF16, kind="ExternalInput")
    whhT_d = nc.dram_tensor("whhT", [H, G3], BF16, kind="ExternalInput")
    bx_d = nc.dram_tensor("bx", [1, G3], BF16, kind="ExternalInput")
    bhn_d = nc.dram_tensor("bhn", [1, 512], BF16, kind="ExternalInput")
    outs_d = nc.dram_tensor("outs", [B, seq_len, H], F32, kind="ExternalOutput")
    hlast_d = nc.dram_tensor("h_last", [B, H], F32, kind="ExternalOutput")

    with tile.TileContext(nc) as tc:
        with tc.tile_pool(name="const", bufs=1) as const:
            ident = const.tile([128, 128], F32, tag="ident")
            make_identity(nc, ident)
            ident_r = const.tile([128, 128], F32R, tag="ident_r")
            nc.vector.tensor_copy(ident_r[:], ident[:])
            ones = const.tile([1, 128], BF16, tag="ones")
            nc.vector.memset(ones[:], 1.0)
            bx_sb = const.tile([1, G3], BF16, tag="bx")
            bhn_sb = const.tile([1, 512], BF16, tag="bhn")
            nc.sync.dma_start(bx_sb[:], bx_d[:])
            nc.sync.dma_start(bhn_sb[:], bhn_d[:])
            wihT = [const.tile([128, G3], BF16, tag=f"wihT{k}", name=f"wihT{k}") for k in range(4)]
            whhT = [const.tile([128, G3], BF16, tag=f"whhT{k}", name=f"whhT{k}") for k in range(4)]
            for k in range(4):
                nc.sync.dma_start(wihT[k][:], wihT_d[k * 128 : (k + 1) * 128, :])
                nc.sync.dma_start(whhT[k][:], whhT_d[k * 128 : (k + 1) * 128, :])

            with (
                tc.tile_pool(name="io", bufs=3) as io_pool,
                tc.tile_pool(name="work", bufs=2) as work,
                tc.tile_pool(name="psg", bufs=1, space="PSUM") as psg,
                tc.tile_pool(name="psh", bufs=2, space="PSUM") as psh,
            ):
                h_cur = work.tile([128, H], F32R, tag="h")
                hT_cur = work.tile([128, H], BF16, tag="hT")
                zero_f32 = work.tile([128, H], F32, tag="z0")
                nc.vector.memset(zero_f32[:], 0.0)
                nc.vector.tensor_copy(h_cur[:], zero_f32[:])
                nc.vector.tensor_copy(hT_cur[:], zero_f32[:])

                def dma_xT(t):
                    xT = io_pool.tile([128, I], BF16, tag="xT", name=f"xT_{t}")
                    nc.sync.dma_start(xT[:], xt_d[t].rearrange("p k b -> p (k b)"))
                    return xT

                def emit_xphase(t, xT):
                    """PSUM alloc + bias matmul + x-side matmuls for step t."""
                    ps_ri0 = psg.tile([128, 512], F32, tag="ri0", name=f"ps_ri0_{t}")
                    ps_ri1 = psg.tile([128, 512], F32, tag="ri1", name=f"ps_ri1_{t}")
                    ps_in = psg.tile([128, 512], F32, tag="inx", name=f"ps_in_{t}", bufs=2)
                    ps_hn = psg.tile([128, 512], F32, tag="hnx", name=f"ps_hn_{t}", bufs=2)
                    nc.tensor.matmul(ps_ri0[:], ones[:], bx_sb[:, 0:512], start=True, stop=False)
                    nc.tensor.matmul(ps_ri1[:], ones[:], bx_sb[:, 512:1024], start=True, stop=False)
                    nc.tensor.matmul(ps_in[:], ones[:], bx_sb[:, 1024:G3], start=True, stop=False)
                    nc.tensor.matmul(ps_hn[:], ones[:], bhn_sb[:], start=True, stop=False)
                    for bank, n0 in ((ps_ri0, 0), (ps_ri1, 512), (ps_in, 1024)):
                        for k in range(4):
                            nc.tensor.matmul(
                                bank[:],
                                xT[:, k * 128 : (k + 1) * 128],
                                wihT[k][:, n0 : n0 + 512],
                                start=False, stop=(n0 == 1024 and k == 3),
                            )
                    return dict(ri0=ps_ri0, ri1=ps_ri1, inx=ps_in, hnx=ps_hn)

                def emit_h_mms(ps, hT):
                    # r bank first (starts its sigmoid earliest), then the
                    # n-gate h bank (t1 = r*gh_n), then i.
                    for bank, n0 in (("ri0", 0), ("hnx", 1024), ("ri1", 512)):
                        for k in range(4):
                            nc.tensor.matmul(
                                ps[bank][:],
                                hT[:, k * 128 : (k + 1) * 128],
                                whhT[k][:, n0 : n0 + 512],
                                start=False, stop=(k == 3),
                            )

                def emit_gates(t, ps, h_prev):
                    r_sb = work.tile([128, 512], F32, tag="r", name=f"r_{t}")
                    i_sb = work.tile([128, 512], F32, tag="ig", name=f"i_{t}")
                    sneg = work.tile([128, 512], F32, tag="sneg", name=f"sneg_{t}")
                    m1 = work.tile([128, 512], F32, tag="m1", name=f"m1_{t}")
                    t1 = work.tile([128, 512], F32, tag="t1", name=f"t1_{t}")
                    t2 = work.tile([128, 512], F32, tag="t2", name=f"t2_{t}")
                    ng = work.tile([128, 512], F32, tag="ng", name=f"ng_{t}")
                    m2 = work.tile([128, 512], F32, tag="m2", name=f"m2_{t}")
                    h_new = work.tile([128, H], F32R, tag="h", name=f"h_{t}")
                    hT_new = work.tile([128, H], BF16, tag="hT", name=f"hT_{t}")
                    ps_h = psh.tile([128, H], F32R, tag="ph", name=f"ps_h_{t}")

                    # ScalarE chain ops, in dependency order.
                    nc.scalar.activation(r_sb[:], ps["ri0"][:], AF.Sigmoid)
                    nc.scalar.activation(i_sb[:], ps["ri1"][:], AF.Sigmoid)
                    # DVE in dependency order: both t1/t2 halves first (tanh
                    # runs on ScalarE), then the post-tanh ops per half.
                    h0, h1 = slice(0, 256), slice(256, 512)
                    nc.vector.tensor_mul(t1[:, h0], r_sb[:, h0], ps["hnx"][:, h0])
                    nc.vector.tensor_add(t2[:, h0], t1[:, h0], ps["inx"][:, h0])
                    nc.vector.tensor_mul(t1[:, h1], r_sb[:, h1], ps["hnx"][:, h1])
                    nc.vector.tensor_add(t2[:, h1], t1[:, h1], ps["inx"][:, h1])
                    # 1 - sigmoid(z) == sigmoid(-z); off ScalarE.
                    nc.vector.tensor_scalar(
                        sneg[:], i_sb[:], -1.0, 1.0,
                        mybir.AluOpType.mult, mybir.AluOpType.add,
                    )
                    nc.gpsimd.tensor_mul(m1[:], i_sb[:], h_prev.bitcast(F32)[:])
                    # Post-tanh tail in 128-col chunks: each h chunk transposes
                    # on the PE as soon as its lerp lands, so the next step's
                    # recurrent matmuls start earlier.
                    for k in range(4):
                        ck = slice(k * 128, (k + 1) * 128)
                        nc.scalar.activation(ng[:, ck], t2[:, ck], AF.Tanh)
                        nc.vector.tensor_mul(m2[:, ck], ng[:, ck], sneg[:, ck])
                        nc.vector.tensor_add(h_new[:, ck], m1[:, ck], m2[:, ck])
                        nc.tensor.transpose(ps_h[:, ck], h_new[:, ck], ident_r[:])
                    nc.scalar.copy(hT_new[:, h0], ps_h[:, h0])
                    nc.scalar.copy(hT_new[:, h1], ps_h[:, h1])
                    nc.sync.dma_start(outs_d[:, t, :], h_new.bitcast(F32)[:])
                    return h_new, hT_new

                xT_cur = dma_xT(0)
                xT_next = dma_xT(1) if seq_len > 1 else None
                ps = emit_xphase(0, xT_cur)
                for t in range(seq_len):
                    emit_h_mms(ps, hT_cur)
                    ps_next = None
                    if t + 1 < seq_len:
                        ps_next = emit_xphase(t + 1, xT_next)
                    if t + 2 < seq_len:
                        xT_cur, xT_next = xT_next, dma_xT(t + 2)
                    h_cur, hT_cur = emit_gates(t, ps, h_cur)
                    ps = ps_next
                nc.sync.dma_start(hlast_d[:], h_cur.bitcast(F32)[:])

    return nc


_BUILT = {}


def get_nc(seq_len=S):
    if seq_len not in _BUILT:
        nc = bacc.Bacc(None, target_bir_lowering=False)
        build(nc, seq_len)
        nc.finalize()
        _BUILT[seq_len] = nc
    return _BUILT[seq_len]


def prep_core_inputs(x_shard, wih, whh, bih, bhh):
    """Host-side preprocessing for one core's input map."""
    seq_len = x_shard.shape[1]
    # xt[t, p, k, b] = x[b, t, 128k+p] in bf16
    xt = np.ascontiguousarray(
        x_shard.astype(BF).transpose(1, 2, 0)  # [S, I, B]
        .reshape(seq_len, 4, 128, x_shard.shape[0])
        .transpose(0, 2, 1, 3)
    )
    bx = np.concatenate([bih[:1024] + bhh[:1024], bih[1024:]]).astype(BF)[None, :]
    bhn = bhh[1024:].astype(BF)[None, :]
    return {
        "xt": xt,
        "wihT": np.ascontiguousarray(wih.T.astype(BF)),
        "whhT": np.ascontiguousarray(whh.T.astype(BF)),
        "bx": np.ascontiguousarray(bx),
        "bhn": np.ascontiguousarray(bhn),
    }


def kernel(x, weight_ih, weight_hh, bias_ih, bias_hh, _trace=False):
    x = np.asarray(x, dtype=np.float32)
    wih = np.asarray(weight_ih, dtype=np.float32)
    whh = np.asarray(weight_hh, dtype=np.float32)
    bih = np.asarray(bias_ih, dtype=np.float32)
    bhh = np.asarray(bias_hh, dtype=np.float32)

    nc = get_nc()
    in_maps = [
        prep_core_inputs(x[c * B : (c + 1) * B], wih, whh, bih, bhh)
        for c in range(NCORES)
    ]
    res = run_bass_kernel_spmd(
        nc, in_maps, core_ids=list(range(NCORES)), trace=_trace
    )
    outs = np.concatenate([r["outs"] for r in res.results], axis=0)
    h_last = np.concatenate([r["h_last"] for r in res.results], axis=0)
    if _trace:
        kernel.last_exec_time_ns = res.exec_time_ns
        kernel.last_results = res
    return outs, h_last


# revision 14
# speedup vs baseline: 1.0469x; 1.0469x over previous
"""Trainium2 Bass kernel for DynamicGRU.

Problem: x [1024, 200, 512] fp32, GRU with H=512.
  gi = x @ W_ih.T + b_ih
  per step: gh = h @ W_hh.T + b_hh
            r = sigmoid(gi_r + gh_r); i = sigmoid(gi_i + gh_i)
            n = tanh(gi_n + r * gh_n)
            h = n + i * (h - n)
Returns (outs [B,S,H], h_last [B,H]).

Data-parallel over 8 cores (128 batch rows each).  Key structure:
  - Matmul operands are bf16 (1 col/cycle PE stream; fp32 and fp32r moving
    operands stream at half rate).  PSUM accumulation is fp32 and the carried
    state h plus gate tensors stay fp32(+fp32r), so only the PE sees bf16.
  - The host pre-transposes x to [S, part, K-chunk, B] bf16 and pre-packs
    W^T bf16 — the kernel never transposes x or W on-chip.
  - gi and gh accumulate into the SAME PSUM bank per gate (r/i), so the
    elementwise adds vanish; the n gate keeps x-side and h-side banks apart
    because r multiplies only the h side.  Biases enter each bank via a K=1
    ones-row outer-product matmul that opens the accumulation group.
  - h is carried as fp32r so its PE transpose (needed to feed the next
    recurrent matmul) runs single-pass; hT is rounded to bf16 in the
    PSUM->SBUF copy.
  - Gate nonlinearities on ScalarE, the rest of the chain on DVE in
    dependency order, m1 = i*h on GPSIMD, all chunked in halves so the PE
    transposes of new h start before the full lerp finishes.
"""

import numpy as np
import ml_dtypes

import concourse.bass as bass
import concourse.mybir as mybir
import concourse.tile as tile
from concourse import bacc
from concourse.bass_utils import run_bass_kernel_spmd
from concourse.masks import make_identity

B_FULL = 1024
NCORES = 8
B = B_FULL // NCORES  # 128 per core
S = 200
I = 512
H = 512
G3 = 3 * H

F32 = mybir.dt.float32
F32R = mybir.dt.float32r
BF16 = mybir.dt.bfloat16
AF = mybir.ActivationFunctionType
BF = ml_dtypes.bfloat16


def build(nc, seq_len=S):
    # Host-prepped inputs: xt[t, p, k, b] = x[b, t, 128k+p] in bf16;
    # wT = W.T (contraction-major) in bf16; biases pre-combined bf16.
    xt_d = nc.dram_tensor("xt", [seq_len, 128, 4, B], BF16, kind="ExternalInput")
    wihT_d = nc.dram_tensor("wihT", [I, G3], BF16, kind="ExternalInput")
    whhT_d = nc.dram_tensor("whhT", [H, G3], BF16, kind="ExternalInput")
    bx_d = nc.dram_tensor("bx", [1, G3], BF16, kind="ExternalInput")
    bhn_d = nc.dram_tensor("bhn", [1, 512], BF16, kind="ExternalInput")
    outs_d = nc.dram_tensor("outs", [B, seq_len, H], F32, kind="ExternalOutput")
    hlast_d = nc.dram_tensor("h_last", [B, H], F32, kind="ExternalOutput")

    with tile.TileContext(nc) as tc:
        with tc.tile_pool(name="const", bufs=1) as const:
            ident = const.tile([128, 128], F32, tag="ident")
            make_identity(nc, ident)
            ident_r = const.tile([128, 128], F32R, tag="ident_r")
            nc.vector.tensor_copy(ident_r[:], ident[:])
            ones = const.tile([1, 128], BF16, tag="ones")
            nc.vector.memset(ones[:], 1.0)
            bx_sb = const.tile([1, G3], BF16, tag="bx")
            bhn_sb = const.tile([1, 512], BF16, tag="bhn")
            nc.sync.dma_start(bx_sb[:], bx_d[:])
            nc.sync.dma_start(bhn_sb[:], bhn_d[:])
            wihT = [const.tile([128, G3], BF16, tag=f"wihT{k}", name=f"wihT{k}") for k in range(4)]
            whhT = [const.tile([128, G3], BF16, tag=f"whhT{k}", name=f"whhT{k}") for k in range(4)]
            for k in range(4):
                nc.sync.dma_start(wihT[k][:], wihT_d[k * 128 : (k + 1) * 128, :])
                nc.sync.dma_start(whhT[k][:], whhT_d[k * 128 : (k + 1) * 128, :])

            with (
                tc.tile_pool(name="io", bufs=3) as io_pool,
                tc.tile_pool(name="work", bufs=2) as work,
                tc.tile_pool(name="psg", bufs=1, space="PSUM") as psg,
                tc.tile_pool(name="psh", bufs=2, space="PSUM") as psh,
            ):
                h_cur = work.tile([128, H], F32R, tag="h")
                hT_cur = work.tile([128, H], BF16, tag="hT")
                zero_f32 = work.tile([128, H], F32, tag="z0")
                nc.vector.memset(zero_f32[:], 0.0)
                nc.vector.tensor_copy(h_cur[:], zero_f32[:])
                nc.vector.tensor_copy(hT_cur[:], zero_f32[:])

                def dma_xT(t):
                    xT = io_pool.tile([128, I], BF16, tag="xT", name=f"xT_{t}")
                    nc.sync.dma_start(xT[:], xt_d[t].rearrange("p k b -> p (k b)"))
                    return xT

                def emit_xphase(t, xT):
                    """PSUM alloc + bias matmul + x-side matmuls for step t."""
                    ps_ri0 = psg.tile([128, 512], F32, tag="ri0", name=f"ps_ri0_{t}")
                    ps_ri1 = psg.tile([128, 512], F32, tag="ri1", name=f"ps_ri1_{t}")
                    ps_in = psg.tile([128, 512], F32, tag="inx", name=f"ps_in_{t}", bufs=2)
                    ps_hn = psg.tile([128, 512], F32, tag="hnx", name=f"ps_hn_{t}", bufs=2)
                    nc.tensor.matmul(ps_ri0[:], ones[:], bx_sb[:, 0:512], start=True, stop=False)
                    nc.tensor.matmul(ps_ri1[:], ones[:], bx_sb[:, 512:1024], start=True, stop=False)
                    nc.tensor.matmul(ps_in[:], ones[:], bx_sb[:, 1024:G3], start=True, stop=False)
                    nc.tensor.matmul(ps_hn[:], ones[:], bhn_sb[:], start=True, stop=False)
                    for bank, n0 in ((ps_ri0, 0), (ps_ri1, 512), (ps_in, 1024)):
                        for k in range(4):
                            nc.tensor.matmul(
                                bank[:],
                                xT[:, k * 128 : (k + 1) * 128],
                                wihT[k][:, n0 : n0 + 512],
                                start=False, stop=(n0 == 1024 and k == 3),
                            )
                    return dict(ri0=ps_ri0, ri1=ps_ri1, inx=ps_in, hnx=ps_hn)

                def emit_h_mms(ps, hT):
                    # r bank first (starts its sigmoid earliest), then the
                    # n-gate h bank (t1 = r*gh_n), then i.
                    for bank, n0 in (("ri0", 0), ("hnx", 1024), ("ri1", 512)):
                        for k in range(4):
                            nc.tensor.matmul(
                                ps[bank][:],
                                hT[:, k * 128 : (k + 1) * 128],
                                whhT[k][:, n0 : n0 + 512],
                                start=False, stop=(k == 3),
                            )

                def emit_gates(t, ps, h_prev):
                    r_sb = work.tile([128, 512], F32, tag="r", name=f"r_{t}")
                    i_sb = work.tile([128, 512], F32, tag="ig", name=f"i_{t}")
                    sneg = work.tile([128, 512], F32, tag="sneg", name=f"sneg_{t}")
                    m1 = work.tile([128, 512], F32, tag="m1", name=f"m1_{t}")
                    t1 = work.tile([128, 512], F32, tag="t1", name=f"t1_{t}")
                    t2 = work.tile([128, 512], F32, tag="t2", name=f"t2_{t}")
                    ng = work.tile([128, 512], F32, tag="ng", name=f"ng_{t}")
                    m2 = work.tile([128, 512], F32, tag="m2", name=f"m2_{t}")
                    h_new = work.tile([128, H], F32R, tag="h", name=f"h_{t}")
                    hT_new = work.tile([128, H], BF16, tag="hT", name=f"hT_{t}")
                    ps_h = psh.tile([128, H], F32R, tag="ph", name=f"ps_h_{t}")

                    # ScalarE chain ops, in dependency order.
                    nc.scalar.activation(r_sb[:], ps["ri0"][:], AF.Sigmoid)
                    nc.scalar.activation(i_sb[:], ps["ri1"][:], AF.Sigmoid)
                    # DVE in dependency order: both t1/t2 halves first (tanh
                    # runs on ScalarE), then the post-tanh ops per half.
                    h0, h1 = slice(0, 256), slice(256, 512)
                    nc.vector.tensor_mul(t1[:, h0], r_sb[:, h0], ps["hnx"][:, h0])
                    nc.vector.tensor_add(t2[:, h0], t1[:, h0], ps["inx"][:, h0])
                    nc.vector.tensor_mul(t1[:, h1], r_sb[:, h1], ps["hnx"][:, h1])
                    nc.vector.tensor_add(t2[:, h1], t1[:, h1], ps["inx"][:, h1])
                    nc.scalar.activation(ng[:, h0], t2[:, h0], AF.Tanh)
                    nc.scalar.activation(ng[:, h1], t2[:, h1], AF.Tanh)
                    # 1 - sigmoid(z) == sigmoid(-z); off ScalarE.
                    nc.vector.tensor_scalar(
                        sneg[:], i_sb[:], -1.0, 1.0,
                        mybir.AluOpType.mult, mybir.AluOpType.add,
                    )
                    nc.gpsimd.tensor_mul(m1[:], i_sb[:], h_prev.bitcast(F32)[:])
                    for c, sl in ((0, h0), (1, h1)):
                        nc.vector.tensor_mul(m2[:, sl], ng[:, sl], sneg[:, sl])
                        nc.vector.tensor_add(h_new[:, sl], m1[:, sl], m2[:, sl])
                        for k in (2 * c, 2 * c + 1):
                            nc.tensor.transpose(
                                ps_h[:, k * 128 : (k + 1) * 128],
                                h_new[:, k * 128 : (k + 1) * 128],
                                ident_r[:],
                            )
                        nc.scalar.copy(hT_new[:, sl], ps_h[:, sl])
                    nc.sync.dma_start(outs_d[:, t, :], h_new.bitcast(F32)[:])
                    return h_new, hT_new

                xT_cur = dma_xT(0)
                xT_next = dma_xT(1) if seq_len > 1 else None
                ps = emit_xphase(0, xT_cur)
                for t in range(seq_len):
                    emit_h_mms(ps, hT_cur)
                    ps_next = None
                    if t + 1 < seq_len:
                        ps_next = emit_xphase(t + 1, xT_next)
                    if t + 2 < seq_len:
                        xT_cur, xT_next = xT_next, dma_xT(t + 2)
                    h_cur, hT_cur = emit_gates(t, ps, h_cur)
                    ps = ps_next
                nc.sync.dma_start(hlast_d[:], h_cur.bitcast(F32)[:])

    return nc


_BUILT = {}


def get_nc(seq_len=S):
    if seq_len not in _BUILT:
        nc = bacc.Bacc(None, target_bir_lowering=False)
        build(nc, seq_len)
        nc.finalize()
        _BUILT[seq_len] = nc
    return _BUILT[seq_len]


def prep_core_inputs(x_shard, wih, whh, bih, bhh):
    """Host-side preprocessing for one core's input map."""
    seq_len = x_shard.shape[1]
    # xt[t, p, k, b] = x[b, t, 128k+p] in bf16
    xt = np.ascontiguousarray(
        x_shard.astype(BF).transpose(1, 2, 0)  # [S, I, B]
        .reshape(seq_len, 4, 128, x_shard.shape[0])
        .transpose(0, 2, 1, 3)
    )
    bx = np.concatenate([bih[:1024] + bhh[:1024], bih[1024:]]).astype(BF)[None, :]
    bhn = bhh[1024:].astype(BF)[None, :]
    return {
        "xt": xt,
        "wihT": np.ascontiguousarray(wih.T.astype(BF)),
        "whhT": np.ascontiguousarray(whh.T.astype(BF)),
        "bx": np.ascontiguousarray(bx),
        "bhn": np.ascontiguousarray(bhn),
    }


def kernel(x, weight_ih, weight_hh, bias_ih, bias_hh, _trace=False):
    x = np.asarray(x, dtype=np.float32)
    wih = np.asarray(weight_ih, dtype=np.float32)
    whh = np.asarray(weight_hh, dtype=np.float32)
    bih = np.asarray(bias_ih, dtype=np.float32)
    bhh = np.asarray(bias_hh, dtype=np.float32)

    nc = get_nc()
    in_maps = [
        prep_core_inputs(x[c * B : (c + 1) * B], wih, whh, bih, bhh)
        for c in range(NCORES)
    ]
    res = run_bass_kernel_spmd(
        nc, in_maps, core_ids=list(range(NCORES)), trace=_trace
    )
    outs = np.concatenate([r["outs"] for r in res.results], axis=0)
    h_last = np.concatenate([r["h_last"] for r in res.results], axis=0)
    if _trace:
        kernel.last_exec_time_ns = res.exec_time_ns
        kernel.last_results = res
    return outs, h_last
